# revision 1
# baseline (speedup 1.0000x reference)
"""Trainium2 Bass kernel for AsymmetricPositionAttentionModule.

Strategy: pure data parallelism — batch B=8 split across 8 NeuronCores, one
image per core. Per-core graph (all matmuls bf16, fp32 accumulate):

  x[512,4096] --cast--> xb (bf16)
  qk  = relu(Wqk'·x + bqk)        2x(8x(4 matmul)) -> ACT relu drain
  val = relu(Wv'·x + bv)          4x(8x(4 matmul))
  key/valPSP = PSP maxpool        VectorE max-tree (scales 6,8 direct; 3,1 derived)
  scoresT = keyT·qk               PE, [110, 4096] PSUM
  esc = exp(scores/16)            ACT, bf16
  sums = onesT·esc                PE broadcast trick -> per-pixel sums on all partitions
  escn = esc * recip(sums)        DVE reciprocal_approx_fast + mult
  W2T = valPSP·Wout'T             PE [110, 512]  (folds out-conv with value)
  z = W2T.T·escn + I·xb           PE, residual via identity matmul
  out = z + bout                  ACT/DVE epilogue, fp32 out
"""

import sys

sys.path.insert(0, "/opt/trn_rl_repo")

from contextlib import ExitStack

import numpy as np
import ml_dtypes

CIN = 512
CK = 256
CV = 512
NPIX = 4096
S = 110
NT = 8          # pixel columns of 512
COL = 512
EPS = 1e-5

_CACHE = {}


def _patch_walrus_flags():
    import concourse.bass_utils as bu

    if getattr(bu, "_ldw_opt_patched", False):
        return
    orig = bu.run_command

    def patched(argv, **kw):
        argv = [
            a
            for a in argv
        ]
        return orig(argv, **kw)

    bu.run_command = patched
    bu._ldw_opt_patched = True


def _build():
    import concourse.bass as bass
    import concourse.tile as tile
    from concourse import bacc, mybir

    _patch_walrus_flags()

    f32 = mybir.dt.float32
    f32r = mybir.dt.float32r
    bf16 = mybir.dt.bfloat16
    ts = bass.ts
    AF = mybir.ActivationFunctionType
    ALU = mybir.AluOpType
    AX = mybir.AxisListType

    nc = bacc.Bacc("TRN2", target_bir_lowering=False, debug=False, num_devices=8)

    x_d = nc.dram_tensor("x", [4, 128, NPIX], bf16, kind="ExternalInput").ap()
    qkw_d = nc.dram_tensor("qk_wt", [4, 128, CK], bf16, kind="ExternalInput").ap()
    vw_d = nc.dram_tensor("v_wt", [4, 128, CV], bf16, kind="ExternalInput").ap()
    w2_d = nc.dram_tensor("w2_rhs", [4, 128, CIN], bf16, kind="ExternalInput").ap()
    bqk_d = nc.dram_tensor("b_qk", [2, 128, 1], f32, kind="ExternalInput").ap()
    bv_d = nc.dram_tensor("b_v", [4, 128, 1], f32, kind="ExternalInput").ap()
    bout_d = nc.dram_tensor("b_out", [4, 128, 1], f32, kind="ExternalInput").ap()
    id_d = nc.dram_tensor("ident", [128, 128], bf16, kind="ExternalInput").ap()
    ones_d = nc.dram_tensor("ones", [S, 128], bf16, kind="ExternalInput").ap()
    out_d = nc.dram_tensor("out", [4, 128, NPIX], bf16, kind="ExternalOutput").ap()

    with tile.TileContext(nc) as tc, ExitStack() as ctx:
        const = ctx.enter_context(tc.tile_pool(name="const", bufs=1))
        persist = ctx.enter_context(tc.tile_pool(name="persist", bufs=1))
        rpool = ctx.enter_context(tc.tile_pool(name="rpool", bufs=2))
        opool = ctx.enter_context(tc.tile_pool(name="opool", bufs=8))
        psum = ctx.enter_context(tc.tile_pool(name="psum", bufs=8, space="PSUM"))

        # ---- constants ----
        wqk = const.tile([128, 4, CK], bf16)
        wv = const.tile([128, 4, CV], bf16)
        w2r = const.tile([128, 4, CIN], bf16)
        bqk = const.tile([128, 2], f32)
        bv = const.tile([128, 4], f32)
        bout = const.tile([128, 4], f32)
        ident = const.tile([128, 128], bf16)
        ones = const.tile([S, 128], bf16)
        # first-conv weights on the scalar queue so the sync queue's first
        # issues are the x chunks the first matmuls wait on.
        for k in range(4):
            nc.scalar.dma_start(wqk[:, k, :], qkw_d[k])
        for m in range(2):
            nc.scalar.dma_start(bqk[:, m : m + 1], bqk_d[m])

        for k in range(4):
            nc.gpsimd.dma_start(wv[:, k, :], vw_d[k])
            nc.gpsimd.dma_start(w2r[:, k, :], w2_d[k])
            nc.gpsimd.dma_start(bv[:, k : k + 1], bv_d[k])
            nc.gpsimd.dma_start(bout[:, k : k + 1], bout_d[k])
        nc.gpsimd.dma_start(ident[:], id_d)
        nc.gpsimd.dma_start(ones[:], ones_d)

        # ---- persistent activations ----
        x32 = persist.tile([128, 4, NPIX], bf16)      # input (bf16), cin chunks
        pin = persist.tile([128, 6, NPIX], bf16)      # qk (blk 0-1) + val (blk 2-5)
        H2 = persist.tile([128, 6, 32, 64], bf16)
        H4 = persist.tile([128, 6, 16, 64], bf16)
        H8 = persist.tile([128, 6, 8, 64], bf16)
        H6 = persist.tile([128, 6, 6, 64], bf16)
        W1 = persist.tile([128, 6, 8, 8, 4], bf16)
        W2s = persist.tile([128, 6, 8, 8, 2], bf16)
        t36 = persist.tile([128, 6, 3, 6], bf16)
        psp = persist.tile([128, 6, S], bf16)         # pooled: [s1|s3|s6|s8]
        esc = persist.tile([S, NPIX], bf16)
        w2t = persist.tile([S, CIN], bf16)

        # views of pin for the pooling tree (free dim = h*64 + w)
        pin_hw = pin.rearrange("p b (h w) -> p b h w", w=64)
        pin_e = pin.rearrange("p b (hp e w) -> p b hp e w", e=2, w=64)
        h2_e = H2.rearrange("p b (hp e) w -> p b hp e w", e=2)
        h4_e = H4.rearrange("p b (hp e) w -> p b hp e w", e=2)
        h8_q = H8.rearrange("p b h (q e f) -> p b h q e f", q=8, e=2, f=4)
        w1_e = W1.rearrange("p b h q (e f) -> p b h q e f", e=2, f=2)
        psp8 = psp[:, :, 46:110].rearrange("p b (i j) -> p b i j", j=8)
        psp6 = psp[:, :, 10:46].rearrange("p b (i j) -> p b i j", j=6)
        psp3 = psp[:, :, 1:10].rearrange("p b (i j) -> p b i j", j=3)
        t36_e = t36.rearrange("p b i (j e) -> p b i j e", e=2)

        def col_htree(blo, bhi, c):
            """per-column h-tree: rows [8c,8c+8) -> H2/H4/H8 for blocks [blo,bhi)."""
            b = slice(blo, bhi)
            nc.vector.tensor_max(
                H2[:, b, 4 * c : 4 * c + 4, :],
                pin_e[:, b, 4 * c : 4 * c + 4, 0, :],
                pin_e[:, b, 4 * c : 4 * c + 4, 1, :],
            )
            nc.vector.tensor_max(
                H4[:, b, 2 * c : 2 * c + 2, :],
                h2_e[:, b, 2 * c : 2 * c + 2, 0, :],
                h2_e[:, b, 2 * c : 2 * c + 2, 1, :],
            )
            nc.vector.tensor_max(
                H8[:, b, c, :],
                h4_e[:, b, c, 0, :],
                h4_e[:, b, c, 1, :],
            )

        def finishing(blo, bhi):
            """s6-h windows, then w-stage for all scales, for blocks [blo,bhi)."""
            b = slice(blo, bhi)
            # s6 h-windows from H2/H4/H8 + single rows
            pieces = [
                [H8[:, b, 0, :], H2[:, b, 4, :], pin_hw[:, b, 10, :]],
                [H2[:, b, 5, :], H4[:, b, 3, :], H4[:, b, 4, :], H2[:, b, 10, :]],
                [pin_hw[:, b, 21, :], H2[:, b, 11, :], H8[:, b, 3, :]],
                [H8[:, b, 4, :], H2[:, b, 20, :], pin_hw[:, b, 42, :]],
                [H2[:, b, 21, :], H4[:, b, 11, :], H4[:, b, 12, :], H2[:, b, 26, :]],
                [pin_hw[:, b, 53, :], H2[:, b, 27, :], H8[:, b, 7, :]],
            ]
            for w, ps in enumerate(pieces):
                dst = H6[:, b, w, :]
                nc.vector.tensor_max(dst, ps[0], ps[1])
                for p in ps[2:]:
                    nc.vector.tensor_max(dst, dst, p)
            # s8 w-tree on H8 [p,b,8h,64w]
            nc.vector.tensor_max(
                W1[:, b], h8_q[:, b, :, :, 0, :], h8_q[:, b, :, :, 1, :]
            )
            nc.vector.tensor_max(
                W2s[:, b], w1_e[:, b, :, :, 0, :], w1_e[:, b, :, :, 1, :]
            )
            nc.vector.tensor_max(
                psp8[:, b], W2s[:, b, :, :, 0], W2s[:, b, :, :, 1]
            )
            # s6 w-windows via reduce over [ws,we)
            for j, (ws, we) in enumerate(
                [(0, 11), (10, 22), (21, 32), (32, 43), (42, 54), (53, 64)]
            ):
                nc.vector.reduce_max(
                    psp6[:, b, :, j], H6[:, b, :, ws:we], axis=AX.X
                )
            # s3 = 2x2 max over s6 grid
            s6i = psp6[:, b].rearrange("p b (i e) j -> p b i e j", e=2)
            nc.vector.tensor_max(t36[:, b], s6i[:, :, :, 0, :], s6i[:, :, :, 1, :])
            nc.vector.tensor_max(
                psp3[:, b], t36_e[:, b, :, :, 0], t36_e[:, b, :, :, 1]
            )
            # s1 = max over s8 cells
            nc.vector.reduce_max(
                psp[:, b, 0:1].rearrange("p b one -> p (b one)"),
                psp8[:, b],
                axis=AX.XY,
            )

        # ---- phase 1: per column-pair: load x, cast (gpsimd), qk conv ----
        for p in range(4):
            p2 = ts(p, 2 * COL)
            if p == 0:
                for k in range(4):
                    nc.sync.dma_start(x32[:, k, ts(0, COL)], x_d[k][:, ts(0, COL)])
                for k in range(4):
                    nc.sync.dma_start(x32[:, k, ts(1, COL)], x_d[k][:, ts(1, COL)])
            else:
                for k in range(4):
                    q = nc.sync if k < 2 else nc.scalar
                    q.dma_start(x32[:, k, p2], x_d[k][:, p2])
            pst = [psum.tile([128, COL], f32, tag="ps", bufs=6, name=f"q{p}{m}") for m in range(4)]
            for m in range(2):
                for k in range(4):
                    for cc in range(2):
                        nc.tensor.matmul(
                            pst[2 * m + cc][:],
                            wqk[:, k, ts(m, 128)],
                            x32[:, k, ts(2 * p + cc, COL)],
                            start=(k == 0),
                            stop=(k == 3),
                            skip_group_check=True,
                        )
            for m in range(2):
                for cc in range(2):
                    nc.scalar.activation(
                        pin[:, m, ts(2 * p + cc, COL)],
                        pst[2 * m + cc][:],
                        AF.Relu,
                        bias=bqk[:, m : m + 1],
                        scale=1.0,
                    )
            col_htree(0, 2, 2 * p)
            col_htree(0, 2, 2 * p + 1)

        # ---- phase 2: qk pooling finish (DVE runs during val convs) ----
        finishing(0, 2)

        # ---- phase 3: val conv m-waves; softmax interleaved after wave 1 ----
        def softmax_block():
            for c in range(NT):
                cs = ts(c, COL)
                ps_s = psum.tile([S, COL], f32, tag="ps", bufs=6, name=f"s{c}")
                for k in range(2):
                    nc.tensor.matmul(
                        ps_s[:],
                        psp[:, k, :],
                        pin[:, k, cs],
                        start=(k == 0),
                        stop=(k == 1),
                        skip_group_check=True,
                    )
                nc.scalar.activation(esc[:, cs], ps_s[:], AF.Exp, scale=0.0625)
                ps_r = psum.tile([128, COL], f32, tag="psr", bufs=2, name=f"r{c}")
                nc.tensor.matmul(ps_r[:], ones[:], esc[:, cs], start=True, stop=True)
                rf = rpool.tile([128, COL], f32, tag="rf")
                nc.vector.reciprocal_approx_fast(rf[:], ps_r[:])
                rb = rpool.tile([128, COL], bf16, tag="rb")
                nc.vector.tensor_copy(rb[:], rf[:])
                nc.vector.tensor_mul(esc[:, cs], esc[:, cs], rb[0:S, :])

        for m in range(4):
            pst = [
                psum.tile([128, COL], f32, tag="ps", bufs=6, name=f"v{m}{c}")
                for c in range(NT)
            ]
            for k in range(4):
                for c in range(NT):
                    nc.tensor.matmul(
                        pst[c][:],
                        wv[:, k, ts(m, 128)],
                        x32[:, k, ts(c, COL)],
                        start=(k == 0),
                        stop=(k == 3),
                        skip_group_check=True,
                    )
            for c in range(NT):
                nc.scalar.activation(
                    pin[:, 2 + m, ts(c, COL)],
                    pst[c][:],
                    AF.Relu,
                    bias=bv[:, m : m + 1],
                    scale=1.0,
                )
            # whole-row h-tree for this val block
            blk = slice(2 + m, 3 + m)
            nc.vector.tensor_max(
                H2[:, blk], pin_e[:, blk, :, 0, :], pin_e[:, blk, :, 1, :]
            )
            nc.vector.tensor_max(
                H4[:, blk], h2_e[:, blk, :, 0, :], h2_e[:, blk, :, 1, :]
            )
            nc.vector.tensor_max(
                H8[:, blk], h4_e[:, blk, :, 0, :], h4_e[:, blk, :, 1, :]
            )
            if m == 0:
                softmax_block()
                finishing(2, 3)
            if m == 1:
                finishing(3, 4)
            if m == 2:
                finishing(4, 5)
                ps_w = psum.tile([S, CIN], f32, tag="psr", bufs=2, name="ps_w")
                for k in range(3):
                    nc.tensor.matmul(
                        ps_w[:],
                        psp[:, 2 + k, :],
                        w2r[:, k, :],
                        start=(k == 0),
                        stop=False,
                        skip_group_check=True,
                    )
            if m == 3:
                finishing(5, 6)

        # ---- phase 4: z waves with W2T folded in ----
        for m in range(4):
            pst = [
                psum.tile([128, COL], f32, tag="ps", bufs=6, name=f"z{m}{c}")
                for c in range(NT)
            ]
            # residual first (only needs x): gives DVE time to finish pooling
            for c in range(NT):
                nc.tensor.matmul(
                    pst[c][:],
                    ident[:],
                    x32[:, m, ts(c, COL)],
                    start=True,
                    stop=False,
                    skip_group_check=True,
                )
            if m == 0:
                nc.tensor.matmul(
                    ps_w[:],
                    psp[:, 5, :],
                    w2r[:, 3, :],
                    start=False,
                    stop=True,
                    skip_group_check=True,
                )
                for mm in range(4):
                    nc.vector.tensor_copy(
                        w2t[:, ts(mm, 128)], ps_w[:, ts(mm, 128)]
                    )
            for c in range(NT):
                nc.tensor.matmul(
                    pst[c][:],
                    w2t[:, ts(m, 128)],
                    esc[:, ts(c, COL)],
                    start=False,
                    stop=True,
                    skip_group_check=True,
                )
            for pp in range(4):
                ot = opool.tile([128, 2 * COL], bf16, tag="ot", name=f"ot{m}{pp}")
                for h in range(2):
                    half = ot[:, ts(h, COL)]
                    zsrc = pst[2 * pp + h][:]
                    if (2 * pp + h) % 2 == 0:
                        nc.scalar.activation(
                            half, zsrc, AF.Identity,
                            bias=bout[:, m : m + 1], scale=1.0,
                        )
                    else:
                        nc.vector.tensor_scalar(
                            half, zsrc, bout[:, m : m + 1], None, ALU.add
                        )
                (nc.sync if pp % 2 == 0 else nc.gpsimd).dma_start(
                    out_d[m][:, ts(pp, 2 * COL)], ot[:]
                )

    nc.compile()
    return nc


def _prep_inputs(inputs):
    def f32a(v):
        return np.asarray(v, dtype=np.float32)

    x = f32a(inputs["x"])
    B = x.shape[0]
    qk_w = f32a(inputs["qk_w"])
    v_w = f32a(inputs["v_w"])
    out_w = f32a(inputs["out_w"])

    def fold(w, gamma, beta, mean, var):
        scale = f32a(gamma) / np.sqrt(f32a(var) + EPS)
        return w * scale[:, None], f32a(beta) - f32a(mean) * scale

    wqk, bqk = fold(qk_w, inputs["qk_gamma"], inputs["qk_beta"], inputs["qk_mean"], inputs["qk_var"])
    wv, bv = fold(v_w, inputs["v_gamma"], inputs["v_beta"], inputs["v_mean"], inputs["v_var"])
    wout, bout = fold(out_w, inputs["out_gamma"], inputs["out_beta"], inputs["out_mean"], inputs["out_var"])

    bf = ml_dtypes.bfloat16
    shared = {
        "qk_wt": np.ascontiguousarray(wqk.T.reshape(4, 128, CK)).astype(bf),
        "v_wt": np.ascontiguousarray(wv.T.reshape(4, 128, CV)).astype(bf),
        "w2_rhs": np.ascontiguousarray(wout.T.reshape(4, 128, CIN)).astype(bf),
        "b_qk": bqk.reshape(2, 128, 1),
        "b_v": bv.reshape(4, 128, 1),
        "b_out": bout.reshape(4, 128, 1),
        "ident": np.eye(128, dtype=np.float32).astype(bf),
        "ones": np.ones((S, 128), dtype=np.float32).astype(bf),
    }
    in_maps = []
    for i in range(B):
        m = dict(shared)
        m["x"] = np.ascontiguousarray(x[i].reshape(4, 128, NPIX)).astype(bf)
        in_maps.append(m)
    return in_maps, x.shape


def _run(inputs, trace=False, trace_kwargs=None):
    from concourse.bass_utils import run_bass_kernel_spmd

    if "nc" not in _CACHE:
        _CACHE["nc"] = _build()
    nc = _CACHE["nc"]
    in_maps, xshape = _prep_inputs(inputs)
    res = run_bass_kernel_spmd(
        nc,
        in_maps,
        core_ids=list(range(len(in_maps))),
        trace=trace,
        **(trace_kwargs or {}),
    )
    B = xshape[0]
    out = np.stack(
        [np.asarray(res.results[i]["out"]).astype(np.float32).reshape(CIN, 64, 64) for i in range(B)]
    )
    return out, res


def kernel(**inputs) -> np.ndarray:
    out, _ = _run(inputs, trace=False)
    return out



# revision 11
# speedup vs baseline: 1.0425x; 1.0425x over previous
"""Trainium2 Bass kernel for AsymmetricPositionAttentionModule.

Strategy: pure data parallelism — batch B=8 split across 8 NeuronCores, one
image per core. Per-core graph (all matmuls bf16, fp32 accumulate):

  x[512,4096] --cast--> xb (bf16)
  qk  = relu(Wqk'·x + bqk)        2x(8x(4 matmul)) -> ACT relu drain
  val = relu(Wv'·x + bv)          4x(8x(4 matmul))
  key/valPSP = PSP maxpool        VectorE max-tree (scales 6,8 direct; 3,1 derived)
  scoresT = keyT·qk               PE, [110, 4096] PSUM
  esc = exp(scores/16)            ACT, bf16
  sums = onesT·esc                PE broadcast trick -> per-pixel sums on all partitions
  escn = esc * recip(sums)        DVE reciprocal_approx_fast + mult
  W2T = valPSP·Wout'T             PE [110, 512]  (folds out-conv with value)
  z = W2T.T·escn + I·xb           PE, residual via identity matmul
  out = z + bout                  ACT/DVE epilogue, fp32 out
"""

import sys

sys.path.insert(0, "/opt/trn_rl_repo")

from contextlib import ExitStack

import numpy as np
import ml_dtypes

CIN = 512
CK = 256
CV = 512
NPIX = 4096
S = 110
NT = 8          # pixel columns of 512
COL = 512
EPS = 1e-5
WSCALE = 1024.0   # pow2 boost for folded conv weights into fp8 range
WUNSCALE = 1.0 / WSCALE

_CACHE = {}


def _patch_walrus_flags():
    import concourse.bass_utils as bu

    if getattr(bu, "_ldw_opt_patched", False):
        return
    orig = bu.run_command

    def patched(argv, **kw):
        argv = [
            a
            for a in argv
        ]
        return orig(argv, **kw)

    bu.run_command = patched
    bu._ldw_opt_patched = True


def _build():
    import concourse.bass as bass
    import concourse.tile as tile
    from concourse import bacc, mybir

    _patch_walrus_flags()

    f32 = mybir.dt.float32
    f32r = mybir.dt.float32r
    bf16 = mybir.dt.bfloat16
    ts = bass.ts
    AF = mybir.ActivationFunctionType
    ALU = mybir.AluOpType
    AX = mybir.AxisListType

    nc = bacc.Bacc("TRN2", target_bir_lowering=False, debug=False, num_devices=8)

    fp8 = mybir.dt.float8e4
    PM = mybir.MatmulPerfMode.DoubleRow

    x_d = nc.dram_tensor("x", [4, 128, NPIX], bf16, kind="ExternalInput").ap()
    x8_d = nc.dram_tensor("x8", [4, 128, NPIX], fp8, kind="ExternalInput").ap()
    qkw_d = nc.dram_tensor("qk_wt", [4, 128, CK], fp8, kind="ExternalInput").ap()
    vw_d = nc.dram_tensor("v_wt", [4, 128, CV], fp8, kind="ExternalInput").ap()
    w2_d = nc.dram_tensor("w2_rhs", [4, 128, CIN], bf16, kind="ExternalInput").ap()
    bqk_d = nc.dram_tensor("b_qk", [2, 128, 1], f32, kind="ExternalInput").ap()
    bv_d = nc.dram_tensor("b_v", [4, 128, 1], f32, kind="ExternalInput").ap()
    bout_d = nc.dram_tensor("b_out", [4, 128, 1], f32, kind="ExternalInput").ap()
    id_d = nc.dram_tensor("ident", [128, 128], bf16, kind="ExternalInput").ap()
    ones_d = nc.dram_tensor("ones", [S, 128], bf16, kind="ExternalInput").ap()
    out_d = nc.dram_tensor("out", [4, 128, NPIX], bf16, kind="ExternalOutput").ap()

    with tile.TileContext(nc) as tc, ExitStack() as ctx:
        const = ctx.enter_context(tc.tile_pool(name="const", bufs=1))
        persist = ctx.enter_context(tc.tile_pool(name="persist", bufs=1))
        rpool = ctx.enter_context(tc.tile_pool(name="rpool", bufs=2))
        opool = ctx.enter_context(tc.tile_pool(name="opool", bufs=8))
        psum = ctx.enter_context(tc.tile_pool(name="psum", bufs=8, space="PSUM"))

        # ---- constants ----
        wqk = const.tile([128, 4, CK], fp8)
        wv = const.tile([128, 4, CV], fp8)
        w2r = const.tile([128, 4, CIN], bf16)
        bqk = const.tile([128, 2], f32)
        bv = const.tile([128, 4], f32)
        bout = const.tile([128, 4], f32)
        ident = const.tile([128, 128], bf16)
        ones = const.tile([S, 128], bf16)
        # first-conv weights on the scalar queue so the sync queue's first
        # issues are the x chunks the first matmuls wait on.
        for k in range(4):
            nc.scalar.dma_start(wqk[:, k, :], qkw_d[k])
        for m in range(2):
            nc.scalar.dma_start(bqk[:, m : m + 1], bqk_d[m])

        for k in range(4):
            nc.gpsimd.dma_start(wv[:, k, :], vw_d[k])
            nc.gpsimd.dma_start(w2r[:, k, :], w2_d[k])
            nc.gpsimd.dma_start(bv[:, k : k + 1], bv_d[k])
            nc.gpsimd.dma_start(bout[:, k : k + 1], bout_d[k])
        nc.gpsimd.dma_start(ident[:], id_d)
        nc.gpsimd.dma_start(ones[:], ones_d)

        # ---- persistent activations ----
        x32 = persist.tile([128, 4, NPIX], bf16)      # input (bf16), residual only
        x8s = persist.tile([128, 4, NPIX], fp8)       # input (fp8), conv operand
        pin = persist.tile([128, 6, NPIX], bf16)      # qk (blk 0-1) + val (blk 2-5)
        H2 = persist.tile([128, 6, 32, 64], bf16)
        H4 = persist.tile([128, 6, 16, 64], bf16)
        H8 = persist.tile([128, 6, 8, 64], bf16)
        H6 = persist.tile([128, 6, 6, 64], bf16)
        W1 = persist.tile([128, 6, 8, 8, 4], bf16)
        W2s = persist.tile([128, 6, 8, 8, 2], bf16)
        t36 = persist.tile([128, 6, 3, 6], bf16)
        psp = persist.tile([128, 6, S], bf16)         # pooled: [s1|s3|s6|s8]
        esc = persist.tile([S, NPIX], bf16)
        w2t = persist.tile([S, CIN], bf16)

        # bf16 x for the residual path only — needed at phase 4, load in the
        # background on the gpsimd queue after the constants.
        for k in range(4):
            nc.gpsimd.dma_start(x32[:, k, :], x_d[k])

        # views of pin for the pooling tree (free dim = h*64 + w)
        pin_hw = pin.rearrange("p b (h w) -> p b h w", w=64)
        pin_e = pin.rearrange("p b (hp e w) -> p b hp e w", e=2, w=64)
        h2_e = H2.rearrange("p b (hp e) w -> p b hp e w", e=2)
        h4_e = H4.rearrange("p b (hp e) w -> p b hp e w", e=2)
        h8_q = H8.rearrange("p b h (q e f) -> p b h q e f", q=8, e=2, f=4)
        w1_e = W1.rearrange("p b h q (e f) -> p b h q e f", e=2, f=2)
        psp8 = psp[:, :, 46:110].rearrange("p b (i j) -> p b i j", j=8)
        psp6 = psp[:, :, 10:46].rearrange("p b (i j) -> p b i j", j=6)
        psp3 = psp[:, :, 1:10].rearrange("p b (i j) -> p b i j", j=3)
        t36_e = t36.rearrange("p b i (j e) -> p b i j e", e=2)

        def col_htree(blo, bhi, c):
            """per-column h-tree: rows [8c,8c+8) -> H2/H4/H8 for blocks [blo,bhi)."""
            b = slice(blo, bhi)
            nc.vector.tensor_max(
                H2[:, b, 4 * c : 4 * c + 4, :],
                pin_e[:, b, 4 * c : 4 * c + 4, 0, :],
                pin_e[:, b, 4 * c : 4 * c + 4, 1, :],
            )
            nc.vector.tensor_max(
                H4[:, b, 2 * c : 2 * c + 2, :],
                h2_e[:, b, 2 * c : 2 * c + 2, 0, :],
                h2_e[:, b, 2 * c : 2 * c + 2, 1, :],
            )
            nc.vector.tensor_max(
                H8[:, b, c, :],
                h4_e[:, b, c, 0, :],
                h4_e[:, b, c, 1, :],
            )

        def finishing(blo, bhi):
            """s6-h windows, then w-stage for all scales, for blocks [blo,bhi)."""
            b = slice(blo, bhi)
            # s6 h-windows from H2/H4/H8 + single rows
            pieces = [
                [H8[:, b, 0, :], H2[:, b, 4, :], pin_hw[:, b, 10, :]],
                [H2[:, b, 5, :], H4[:, b, 3, :], H4[:, b, 4, :], H2[:, b, 10, :]],
                [pin_hw[:, b, 21, :], H2[:, b, 11, :], H8[:, b, 3, :]],
                [H8[:, b, 4, :], H2[:, b, 20, :], pin_hw[:, b, 42, :]],
                [H2[:, b, 21, :], H4[:, b, 11, :], H4[:, b, 12, :], H2[:, b, 26, :]],
                [pin_hw[:, b, 53, :], H2[:, b, 27, :], H8[:, b, 7, :]],
            ]
            for w, ps in enumerate(pieces):
                dst = H6[:, b, w, :]
                nc.vector.tensor_max(dst, ps[0], ps[1])
                for p in ps[2:]:
                    nc.vector.tensor_max(dst, dst, p)
            # s8 w-tree on H8 [p,b,8h,64w]
            nc.vector.tensor_max(
                W1[:, b], h8_q[:, b, :, :, 0, :], h8_q[:, b, :, :, 1, :]
            )
            nc.vector.tensor_max(
                W2s[:, b], w1_e[:, b, :, :, 0, :], w1_e[:, b, :, :, 1, :]
            )
            nc.vector.tensor_max(
                psp8[:, b], W2s[:, b, :, :, 0], W2s[:, b, :, :, 1]
            )
            # s6 w-windows via reduce over [ws,we)
            for j, (ws, we) in enumerate(
                [(0, 11), (10, 22), (21, 32), (32, 43), (42, 54), (53, 64)]
            ):
                nc.vector.reduce_max(
                    psp6[:, b, :, j], H6[:, b, :, ws:we], axis=AX.X
                )
            # s3 = 2x2 max over s6 grid
            s6i = psp6[:, b].rearrange("p b (i e) j -> p b i e j", e=2)
            nc.vector.tensor_max(t36[:, b], s6i[:, :, :, 0, :], s6i[:, :, :, 1, :])
            nc.vector.tensor_max(
                psp3[:, b], t36_e[:, b, :, :, 0], t36_e[:, b, :, :, 1]
            )
            # s1 = max over s8 cells
            nc.vector.reduce_max(
                psp[:, b, 0:1].rearrange("p b one -> p (b one)"),
                psp8[:, b],
                axis=AX.XY,
            )

        # ---- phase 1: per column-pair: load x8, qk conv (fp8 DoubleRow) ----
        for p in range(4):
            p2 = ts(p, 2 * COL)
            if p == 0:
                for k in range(4):
                    nc.sync.dma_start(x8s[:, k, ts(0, COL)], x8_d[k][:, ts(0, COL)])
                for k in range(4):
                    nc.sync.dma_start(x8s[:, k, ts(1, COL)], x8_d[k][:, ts(1, COL)])
            else:
                for k in range(4):
                    q = nc.sync if k < 2 else nc.scalar
                    q.dma_start(x8s[:, k, p2], x8_d[k][:, p2])
            pst = [psum.tile([128, COL], f32, tag="ps", bufs=6, name=f"q{p}{m}") for m in range(4)]
            for m in range(2):
                for kp in range(2):
                    for cc in range(2):
                        nc.tensor.matmul(
                            pst[2 * m + cc][:],
                            wqk[:, 2 * kp : 2 * kp + 2, ts(m, 128)],
                            x8s[:, 2 * kp : 2 * kp + 2, ts(2 * p + cc, COL)],
                            start=(kp == 0),
                            stop=(kp == 1),
                            perf_mode=PM,
                            skip_group_check=True,
                        )
            for m in range(2):
                for cc in range(2):
                    nc.scalar.activation(
                        pin[:, m, ts(2 * p + cc, COL)],
                        pst[2 * m + cc][:],
                        AF.Relu,
                        bias=bqk[:, m : m + 1],
                        scale=WUNSCALE,
                    )
            col_htree(0, 2, 2 * p)
            col_htree(0, 2, 2 * p + 1)

        # ---- phase 2: qk pooling finish (DVE runs during val convs) ----
        finishing(0, 2)

        # ---- phase 3: val conv m-waves; softmax interleaved after wave 1 ----
        def softmax_block():
            for c in range(NT):
                cs = ts(c, COL)
                ps_s = psum.tile([S, COL], f32, tag="ps", bufs=6, name=f"s{c}")
                for k in range(2):
                    nc.tensor.matmul(
                        ps_s[:],
                        psp[:, k, :],
                        pin[:, k, cs],
                        start=(k == 0),
                        stop=(k == 1),
                        skip_group_check=True,
                    )
                nc.scalar.activation(esc[:, cs], ps_s[:], AF.Exp, scale=0.0625)
                ps_r = psum.tile([128, COL], f32, tag="psr", bufs=2, name=f"r{c}")
                nc.tensor.matmul(ps_r[:], ones[:], esc[:, cs], start=True, stop=True)
                rf = rpool.tile([128, COL], f32, tag="rf")
                nc.vector.reciprocal_approx_fast(rf[:], ps_r[:])
                rb = rpool.tile([128, COL], bf16, tag="rb")
                nc.vector.tensor_copy(rb[:], rf[:])
                nc.vector.tensor_mul(esc[:, cs], esc[:, cs], rb[0:S, :])

        for m in range(4):
            pst = [
                psum.tile([128, COL], f32, tag="ps", bufs=6, name=f"v{m}{c}")
                for c in range(NT)
            ]
            for kp in range(2):
                for c in range(NT):
                    nc.tensor.matmul(
                        pst[c][:],
                        wv[:, 2 * kp : 2 * kp + 2, ts(m, 128)],
                        x8s[:, 2 * kp : 2 * kp + 2, ts(c, COL)],
                        start=(kp == 0),
                        stop=(kp == 1),
                        perf_mode=PM,
                        skip_group_check=True,
                    )
            for c in range(NT):
                nc.scalar.activation(
                    pin[:, 2 + m, ts(c, COL)],
                    pst[c][:],
                    AF.Relu,
                    bias=bv[:, m : m + 1],
                    scale=WUNSCALE,
                )
            # whole-row h-tree for this val block
            blk = slice(2 + m, 3 + m)
            nc.vector.tensor_max(
                H2[:, blk], pin_e[:, blk, :, 0, :], pin_e[:, blk, :, 1, :]
            )
            nc.vector.tensor_max(
                H4[:, blk], h2_e[:, blk, :, 0, :], h2_e[:, blk, :, 1, :]
            )
            nc.vector.tensor_max(
                H8[:, blk], h4_e[:, blk, :, 0, :], h4_e[:, blk, :, 1, :]
            )
            if m == 0:
                softmax_block()
                finishing(2, 3)
            if m == 1:
                finishing(3, 4)
            if m == 2:
                finishing(4, 5)
                ps_w = psum.tile([S, CIN], f32, tag="psr", bufs=2, name="ps_w")
                for k in range(3):
                    nc.tensor.matmul(
                        ps_w[:],
                        psp[:, 2 + k, :],
                        w2r[:, k, :],
                        start=(k == 0),
                        stop=False,
                        skip_group_check=True,
                    )
            if m == 3:
                finishing(5, 6)

        # ---- phase 4: z waves with W2T folded in ----
        for m in range(4):
            pst = [
                psum.tile([128, COL], f32, tag="ps", bufs=6, name=f"z{m}{c}")
                for c in range(NT)
            ]
            # residual first (only needs x): gives DVE time to finish pooling
            for c in range(NT):
                nc.tensor.matmul(
                    pst[c][:],
                    ident[:],
                    x32[:, m, ts(c, COL)],
                    start=True,
                    stop=False,
                    skip_group_check=True,
                )
            if m == 0:
                nc.tensor.matmul(
                    ps_w[:],
                    psp[:, 5, :],
                    w2r[:, 3, :],
                    start=False,
                    stop=True,
                    skip_group_check=True,
                )
                for mm in range(4):
                    nc.vector.tensor_copy(
                        w2t[:, ts(mm, 128)], ps_w[:, ts(mm, 128)]
                    )
            for c in range(NT):
                nc.tensor.matmul(
                    pst[c][:],
                    w2t[:, ts(m, 128)],
                    esc[:, ts(c, COL)],
                    start=False,
                    stop=True,
                    skip_group_check=True,
                )
            for pp in range(4):
                ot = opool.tile([128, 2 * COL], bf16, tag="ot", name=f"ot{m}{pp}")
                for h in range(2):
                    half = ot[:, ts(h, COL)]
                    zsrc = pst[2 * pp + h][:]
                    if (2 * pp + h) % 2 == 0:
                        nc.scalar.activation(
                            half, zsrc, AF.Identity,
                            bias=bout[:, m : m + 1], scale=1.0,
                        )
                    else:
                        nc.vector.tensor_scalar(
                            half, zsrc, bout[:, m : m + 1], None, ALU.add
                        )
                (nc.sync if pp % 2 == 0 else nc.gpsimd).dma_start(
                    out_d[m][:, ts(pp, 2 * COL)], ot[:]
                )

    nc.compile()
    return nc


def _prep_inputs(inputs):
    def f32a(v):
        return np.asarray(v, dtype=np.float32)

    x = f32a(inputs["x"])
    B = x.shape[0]
    qk_w = f32a(inputs["qk_w"])
    v_w = f32a(inputs["v_w"])
    out_w = f32a(inputs["out_w"])

    def fold(w, gamma, beta, mean, var):
        scale = f32a(gamma) / np.sqrt(f32a(var) + EPS)
        return w * scale[:, None], f32a(beta) - f32a(mean) * scale

    wqk, bqk = fold(qk_w, inputs["qk_gamma"], inputs["qk_beta"], inputs["qk_mean"], inputs["qk_var"])
    wv, bv = fold(v_w, inputs["v_gamma"], inputs["v_beta"], inputs["v_mean"], inputs["v_var"])
    wout, bout = fold(out_w, inputs["out_gamma"], inputs["out_beta"], inputs["out_mean"], inputs["out_var"])

    bf = ml_dtypes.bfloat16
    f8 = ml_dtypes.float8_e4m3

    def to_f8(a):
        return np.clip(a, -240.0, 240.0).astype(f8)

    shared = {
        "qk_wt": to_f8(np.ascontiguousarray(wqk.T.reshape(4, 128, CK)) * WSCALE),
        "v_wt": to_f8(np.ascontiguousarray(wv.T.reshape(4, 128, CV)) * WSCALE),
        "w2_rhs": np.ascontiguousarray(wout.T.reshape(4, 128, CIN)).astype(bf),
        "b_qk": bqk.reshape(2, 128, 1),
        "b_v": bv.reshape(4, 128, 1),
        "b_out": bout.reshape(4, 128, 1),
        "ident": np.eye(128, dtype=np.float32).astype(bf),
        "ones": np.ones((S, 128), dtype=np.float32).astype(bf),
    }
    in_maps = []
    for i in range(B):
        m = dict(shared)
        xi = np.ascontiguousarray(x[i].reshape(4, 128, NPIX))
        m["x"] = xi.astype(bf)
        m["x8"] = to_f8(xi)
        in_maps.append(m)
    return in_maps, x.shape


def _run(inputs, trace=False, trace_kwargs=None):
    from concourse.bass_utils import run_bass_kernel_spmd

    if "nc" not in _CACHE:
        _CACHE["nc"] = _build()
    nc = _CACHE["nc"]
    in_maps, xshape = _prep_inputs(inputs)
    res = run_bass_kernel_spmd(
        nc,
        in_maps,
        core_ids=list(range(len(in_maps))),
        trace=trace,
        **(trace_kwargs or {}),
    )
    B = xshape[0]
    out = np.stack(
        [np.asarray(res.results[i]["out"]).astype(np.float32).reshape(CIN, 64, 64) for i in range(B)]
    )
    return out, res


def kernel(**inputs) -> np.ndarray:
    out, _ = _run(inputs, trace=False)
    return out

